# revision 4
# baseline (speedup 1.0000x reference)
"""Trainium2 Bass kernel for a pre-norm transformer encoder block.

Reference computation (per batch):
    x = x + MHA(LN1(x));  x = x + FFN(LN2(x))
with B=2, S=2048, D=1024, H=16 heads (HD=64), HID=4096, fp32 params,
src_mask all-ones (no-op).

Sharding: pure data parallel over the 8 NeuronCores. Core c handles batch
b = c // 4 and query-token chunk c % 4 (512 tokens). Each core recomputes
K/V for its full batch (4x redundant) so no collectives are needed. The
batch rows are rolled on the host so each core's own tokens are rows 0:512;
attention is permutation-invariant over keys so rolling is safe.

On-chip math: all GEMMs in bf16 with fp32 PSUM accumulation; layernorm,
softmax statistics and residuals in fp32. Softmax is computed in
"transposed score" space (keys on partitions) so no P-matrix transposes
are needed; the row sums come from an extra all-ones column appended to V,
and the 1/sum normalization is applied to the attention output via a
DRAM-bounce broadcast.
"""

import numpy as np
import ml_dtypes

import concourse.bacc as bacc
import concourse.bass as bass
import concourse.mybir as mybir
import concourse.tile as tile
from concourse.masks import make_identity

P = 128
B, S, D, H, HD, HID = 2, 2048, 1024, 16, 64, 4096
T = 512                     # own query tokens per core
DC = D // P                 # 8  d-chunks
SC = S // P                 # 16 key-chunks
TC = T // P                 # 4  own-token chunks
RC = HID // P               # 32 hidden chunks
NCORES = 8
EPS = 1e-5

F32 = mybir.dt.float32
BF16 = mybir.dt.bfloat16
AF = mybir.ActivationFunctionType
ALU = mybir.AluOpType
BF_NP = ml_dtypes.bfloat16


def _build_nc():
    nc = bacc.Bacc("TRN2", target_bir_lowering=False, debug=False)

    xb = nc.declare_dram_parameter("xb", [S, D], F32, isOutput=False)
    wqT = nc.declare_dram_parameter("wqT", [D, D], BF16, isOutput=False)
    wkT = nc.declare_dram_parameter("wkT", [D, D], BF16, isOutput=False)
    wvT = nc.declare_dram_parameter("wvT", [D, D], BF16, isOutput=False)
    woT = nc.declare_dram_parameter("woT", [D, D], BF16, isOutput=False)
    w1 = nc.declare_dram_parameter("w1", [D, HID], BF16, isOutput=False)
    w2 = nc.declare_dram_parameter("w2", [HID, D], BF16, isOutput=False)
    ln1g = nc.declare_dram_parameter("ln1g", [D], F32, isOutput=False)
    ln1b = nc.declare_dram_parameter("ln1b", [D], F32, isOutput=False)
    ln2g = nc.declare_dram_parameter("ln2g", [D], F32, isOutput=False)
    ln2b = nc.declare_dram_parameter("ln2b", [D], F32, isOutput=False)
    b1 = nc.declare_dram_parameter("b1", [HID], F32, isOutput=False)
    b2 = nc.declare_dram_parameter("b2", [D], F32, isOutput=False)
    out = nc.declare_dram_parameter("out", [T, D], F32, isOutput=True)

    sums_dram = nc.dram_tensor("sums_dram", [H, T], F32)
    recip_dram = nc.dram_tensor("recip_dram", [H, T], F32)

    # DRAM views used for strided loads
    wqT_r = wqT[:, :].rearrange("(dc p) o -> p dc o", p=P)
    wkT_r = wkT[:, :].rearrange("(dc p) o -> p dc o", p=P)
    wvT_r = wvT[:, :].rearrange("(dc p) o -> p dc o", p=P)
    woT_r = woT[:, :].rearrange("(dc p) o -> p dc o", p=P)
    w1_r = w1[:, :].rearrange("(dc p) r -> p dc r", p=P)
    w2_r = w2[:, :].rearrange("(rc p) d -> p rc d", p=P)

    def bcast_rows(src_ap, nrows):
        return bass.AP(tensor=src_ap.tensor, offset=src_ap.offset,
                       ap=[[0, nrows], *src_ap.ap[1:]])

    import contextlib
    with tile.TileContext(nc) as tc, contextlib.ExitStack() as ctx:
        consts = ctx.enter_context(tc.tile_pool(name="consts", bufs=1))
        persist = ctx.enter_context(tc.tile_pool(name="persist", bufs=1))
        poolA = ctx.enter_context(tc.tile_pool(name="poolA", bufs=1))
        poolB = ctx.enter_context(tc.tile_pool(name="poolB", bufs=1))
        small = ctx.enter_context(tc.tile_pool(name="small", bufs=4))
        xb_pool = ctx.enter_context(tc.tile_pool(name="xb_pool", bufs=2))
        wsmall = ctx.enter_context(tc.tile_pool(name="wsmall", bufs=2))
        wbig = ctx.enter_context(tc.tile_pool(name="wbig", bufs=1))
        wstream = ctx.enter_context(tc.tile_pool(name="wstream", bufs=3))
        exp_pool = ctx.enter_context(tc.tile_pool(name="exp_pool", bufs=6))
        sums_pool = ctx.enter_context(tc.tile_pool(name="sums_pool", bufs=2))
        rdup_pool = ctx.enter_context(tc.tile_pool(name="rdup_pool", bufs=2))
        out_pool = ctx.enter_context(tc.tile_pool(name="out_pool", bufs=2))

        # ---------------- constants ----------------
        identity = consts.tile([P, P], BF16)
        make_identity(nc, identity)
        eps_t = consts.tile([P, 1], F32)
        nc.vector.memset(eps_t, EPS)
        g1_sb = consts.tile([P, DC], F32)
        nc.sync.dma_start(out=g1_sb, in_=ln1g[:].rearrange("(c p) -> p c", p=P))
        b1ln_sb = consts.tile([P, DC], F32)
        nc.sync.dma_start(out=b1ln_sb, in_=ln1b[:].rearrange("(c p) -> p c", p=P))
        g2_sb = consts.tile([P, DC], F32)
        nc.sync.dma_start(out=g2_sb, in_=ln2g[:].rearrange("(c p) -> p c", p=P))
        b2ln_sb = consts.tile([P, DC], F32)
        nc.sync.dma_start(out=b2ln_sb, in_=ln2b[:].rearrange("(c p) -> p c", p=P))
        b1_sb = consts.tile([P, RC], F32)
        nc.sync.dma_start(out=b1_sb, in_=b1[:].rearrange("(c p) -> p c", p=P))
        b2rep = consts.tile([P, D], F32)
        nc.sync.dma_start(out=b2rep, in_=bcast_rows(b2[:].rearrange("(one d) -> one d", one=1), P))

        # ---------------- persistent tensors ----------------
        x_own = persist.tile([P, TC, D], F32)       # own x rows; becomes x2 in place
        QT = persist.tile([P, DC, T], BF16)
        KT = persist.tile([P, DC, S], BF16)
        attnT = persist.tile([P, DC, T], BF16)
        hT = persist.tile([P, DC, T], BF16)
        h_bf = persist.tile([P, TC, D], BF16)
        xn_bf = poolA.tile([P, SC, D], BF16, tag="sharedA", name="xn_bf")

        def layernorm_chunk(src, dst_bf, g_unused=None):
            """src [P, D] f32 -> dst_bf [P, D] bf16 normalized (no gamma/beta)."""
            stats = small.tile([P, 2, 6], F32, tag="stats", name="stats")
            nc.vector.bn_stats(out=stats[:, 0, :], in_=src[:, 0:512])
            nc.vector.bn_stats(out=stats[:, 1, :], in_=src[:, 512:1024])
            mv = small.tile([P, 2], F32, tag="mv", name="mv")
            nc.vector.bn_aggr(out=mv, in_=stats)
            std = small.tile([P, 1], F32, tag="std", name="std")
            nc.scalar.activation(out=std, in_=mv[:, 1:2], func=AF.Sqrt, bias=eps_t)
            rstd = small.tile([P, 1], F32, tag="rstd", name="rstd")
            nc.vector.reciprocal(out=rstd, in_=std)
            nc.vector.tensor_scalar(out=dst_bf, in0=src, scalar1=mv[:, 0:1],
                                    scalar2=rstd, op0=ALU.subtract, op1=ALU.mult)

        # ================ Phase 1: LN1 + transpose + QKV ================
        with tc.tile_pool(name="pt", bufs=2, space="PSUM") as pt, \
             tc.tile_pool(name="pq", bufs=3, space="PSUM") as pq:

            # LN1 over the full batch (16 chunks); own rows also kept in f32
            for t in range(SC):
                if t < TC:
                    xt = x_own[:, t, :]
                else:
                    xtile = xb_pool.tile([P, D], F32, tag="xb", name=f"xb_{t}")
                    xt = xtile
                nc.sync.dma_start(out=xt, in_=xb[t * P:(t + 1) * P, :])
                layernorm_chunk(xt, xn_bf[:, t, :])

            # transpose xn -> xnT, applying ln1 gamma/beta on the drain
            xnT = poolB.tile([P, DC, S], BF16, tag="sharedB", name="xnT")
            for dc in range(DC):
                for tq in range(SC // 4):
                    ps_t = pt.tile([P, 4, P], BF16, tag="tp", name=f"tp_{dc}_{tq}")
                    for i in range(4):
                        nc.tensor.transpose(
                            ps_t[:, i, :], xn_bf[:, tq * 4 + i, dc * P:(dc + 1) * P],
                            identity)
                    dst = xnT[:, dc, tq * 512:(tq + 1) * 512]
                    nc.vector.tensor_scalar(
                        out=dst.rearrange("p (i c) -> p i c", i=4),
                        in0=ps_t, scalar1=g1_sb[:, dc:dc + 1],
                        scalar2=b1ln_sb[:, dc:dc + 1],
                        op0=ALU.mult, op1=ALU.add)

            # Q^T projection: [P, DC, T]
            for oc in range(DC):
                wq_t = wsmall.tile([P, DC, P], BF16, tag="wq", name=f"wq_{oc}")
                nc.sync.dma_start(out=wq_t, in_=wqT_r[:, :, oc * P:(oc + 1) * P])
                ps = pq.tile([P, T], F32, tag="qkv", name=f"psq_{oc}")
                for dc in range(DC):
                    nc.tensor.matmul(ps, lhsT=wq_t[:, dc, :], rhs=xnT[:, dc, 0:T],
                                     start=(dc == 0), stop=(dc == DC - 1))
                nc.scalar.copy(out=QT[:, oc, :], in_=ps)

            # K^T projection: [P, DC, S]
            for oc in range(DC):
                wk_t = wsmall.tile([P, DC, P], BF16, tag="wk", name=f"wk_{oc}")
                nc.sync.dma_start(out=wk_t, in_=wkT_r[:, :, oc * P:(oc + 1) * P])
                for nt in range(S // 512):
                    ps = pq.tile([P, 512], F32, tag="qkv", name=f"psk_{oc}_{nt}")
                    for dc in range(DC):
                        nc.tensor.matmul(
                            ps, lhsT=wk_t[:, dc, :],
                            rhs=xnT[:, dc, nt * 512:(nt + 1) * 512],
                            start=(dc == 0), stop=(dc == DC - 1))
                    nc.scalar.copy(out=KT[:, oc, nt * 512:(nt + 1) * 512], in_=ps)

            # V natural projection with ones column: [P, SC, H, HD+1]
            V_sb = poolA.tile([P, SC, H, HD + 1], BF16, tag="sharedA", name="V_sb")
            nc.vector.memset(V_sb[:, :, :, HD:HD + 1], 1.0)
            for jn in range(2):
                wv_t = wbig.tile([P, DC, 512], BF16, tag="wv", name=f"wv_{jn}")
                nc.sync.dma_start(out=wv_t, in_=wvT_r[:, :, jn * 512:(jn + 1) * 512])
                for sc in range(SC):
                    ps = pq.tile([P, 512], F32, tag="qkv", name=f"psv_{jn}_{sc}")
                    for dc in range(DC):
                        nc.tensor.matmul(
                            ps, lhsT=xnT[:, dc, sc * P:(sc + 1) * P],
                            rhs=wv_t[:, dc, :],
                            start=(dc == 0), stop=(dc == DC - 1))
                    nc.scalar.copy(
                        out=V_sb[:, sc, jn * 8:(jn + 1) * 8, 0:HD],
                        in_=ps.rearrange("p (h d) -> p h d", h=8))

        # ================ Phase 2: attention ================
        with tc.tile_pool(name="psc", bufs=4, space="PSUM") as psc, \
             tc.tile_pool(name="ppv", bufs=4, space="PSUM") as ppv:

            for p8 in range(H // 2):
                hA, hB = 2 * p8, 2 * p8 + 1
                pvA = ppv.tile([HD + 1, T], F32, tag="pv", name=f"pvA_{p8}")
                pvB = ppv.tile([HD + 1, T], F32, tag="pv", name=f"pvB_{p8}")
                for kc in range(SC):
                    sA = psc.tile([P, T], F32, tag="sc", name=f"sA_{p8}_{kc}")
                    sB = psc.tile([P, T], F32, tag="sc", name=f"sB_{p8}_{kc}")
                    nc.tensor.matmul(sA, lhsT=KT[0:64, p8, kc * P:(kc + 1) * P],
                                     rhs=QT[0:64, p8, :], start=True, stop=True,
                                     tile_position=(0, 0))
                    nc.tensor.matmul(sB, lhsT=KT[64:128, p8, kc * P:(kc + 1) * P],
                                     rhs=QT[64:128, p8, :], start=True, stop=True,
                                     tile_position=(64, 0))
                    eA = exp_pool.tile([P, T], BF16, tag="exp", name=f"eA_{p8}_{kc}")
                    eB = exp_pool.tile([P, T], BF16, tag="exp", name=f"eB_{p8}_{kc}")
                    nc.scalar.activation(out=eA, in_=sA, func=AF.Exp, scale=0.125)
                    nc.scalar.activation(out=eB, in_=sB, func=AF.Exp, scale=0.125)
                    nc.tensor.matmul(pvA, lhsT=V_sb[:, kc, hA, :], rhs=eA,
                                     start=(kc == 0), stop=(kc == SC - 1))
                    nc.tensor.matmul(pvB, lhsT=V_sb[:, kc, hB, :], rhs=eB,
                                     start=(kc == 0), stop=(kc == SC - 1))
                # sums row -> DRAM bounce; unnormalized attnT -> SBUF (bf16)
                smA = sums_pool.tile([1, T], F32, tag="sums", name=f"smA_{p8}")
                smB = sums_pool.tile([1, T], F32, tag="sums", name=f"smB_{p8}")
                nc.vector.tensor_copy(out=smA, in_=pvA[HD:HD + 1, :])
                nc.vector.tensor_copy(out=smB, in_=pvB[HD:HD + 1, :])
                nc.sync.dma_start(out=sums_dram[hA:hA + 1, :], in_=smA)
                nc.sync.dma_start(out=sums_dram[hB:hB + 1, :], in_=smB)
                nc.vector.tensor_copy(out=attnT[0:64, p8, :], in_=pvA[0:HD, :])
                nc.vector.tensor_copy(out=attnT[64:128, p8, :], in_=pvB[0:HD, :])

            # reciprocal of all head sums, then broadcast-normalize attnT
            sums16 = persist.tile([H, T], F32)
            nc.sync.dma_start(out=sums16, in_=sums_dram[:, :])
            recip16 = persist.tile([H, T], F32)
            nc.vector.reciprocal(out=recip16, in_=sums16)
            nc.sync.dma_start(out=recip_dram[:, :], in_=recip16)
            for p8 in range(H // 2):
                rd = rdup_pool.tile([P, T], F32, tag="rdup", name=f"rd_{p8}")
                nc.sync.dma_start(out=rd[0:64, :],
                                  in_=bcast_rows(recip_dram[2 * p8:2 * p8 + 1, :], 64))
                nc.sync.dma_start(out=rd[64:128, :],
                                  in_=bcast_rows(recip_dram[2 * p8 + 1:2 * p8 + 2, :], 64))
                nc.vector.tensor_tensor(out=attnT[:, p8, :], in0=attnT[:, p8, :],
                                        in1=rd, op=ALU.mult)

        # ================ Phase 3: output proj + residual + LN2 ================
        with tc.tile_pool(name="po", bufs=5, space="PSUM") as po, \
             tc.tile_pool(name="pt2", bufs=2, space="PSUM") as pt2:

            for jn in range(2):
                pss = [po.tile([P, 512], F32, tag="o", name=f"pso_{jn}_{t_}")
                       for t_ in range(TC)]
                for ic in range(DC):
                    wo_t = wstream.tile([P, 512], BF16, tag="wo", name=f"wo_{jn}_{ic}")
                    nc.sync.dma_start(out=wo_t, in_=woT_r[:, ic, jn * 512:(jn + 1) * 512])
                    for t_ in range(TC):
                        nc.tensor.matmul(pss[t_], lhsT=attnT[:, ic, t_ * P:(t_ + 1) * P],
                                         rhs=wo_t, start=(ic == 0), stop=(ic == DC - 1))
                for t_ in range(TC):
                    sl = x_own[:, t_, jn * 512:(jn + 1) * 512]
                    nc.vector.tensor_tensor(out=sl, in0=pss[t_], in1=sl, op=ALU.add)

            # LN2 (x_own now holds x2); h_bf bf16 normalized
            for t_ in range(TC):
                layernorm_chunk(x_own[:, t_, :], h_bf[:, t_, :])
                # after LN2 consumed the chunk, fold b2 into the residual base
                nc.vector.tensor_tensor(out=x_own[:, t_, :], in0=x_own[:, t_, :],
                                        in1=b2rep, op=ALU.add)

            # transpose h -> hT with ln2 gamma/beta fused on the drain
            for dc in range(DC):
                ps_t = pt2.tile([P, TC, P], BF16, tag="tp2", name=f"tp2_{dc}")
                for i in range(TC):
                    nc.tensor.transpose(
                        ps_t[:, i, :], h_bf[:, i, dc * P:(dc + 1) * P], identity)
                nc.vector.tensor_scalar(
                    out=hT[:, dc, :].rearrange("p (i c) -> p i c", i=TC),
                    in0=ps_t, scalar1=g2_sb[:, dc:dc + 1],
                    scalar2=b2ln_sb[:, dc:dc + 1],
                    op0=ALU.mult, op1=ALU.add)

        # ================ Phase 4: FFN ================
        with tc.tile_pool(name="pf1", bufs=3, space="PSUM") as pf1, \
             tc.tile_pool(name="pf2", bufs=5, space="PSUM") as pf2:

            h1T = poolB.tile([P, RC, T], BF16, tag="sharedB", name="h1T")
            for rc in range(RC):
                w1_t = wsmall.tile([P, DC, P], BF16, tag="w1", name=f"w1_{rc}")
                nc.sync.dma_start(out=w1_t, in_=w1_r[:, :, rc * P:(rc + 1) * P])
                ps = pf1.tile([P, T], F32, tag="f1", name=f"psf1_{rc}")
                for dc in range(DC):
                    nc.tensor.matmul(ps, lhsT=w1_t[:, dc, :], rhs=hT[:, dc, :],
                                     start=(dc == 0), stop=(dc == DC - 1))
                # relu(x + b1) fused on the drain
                nc.vector.tensor_scalar(out=h1T[:, rc, :], in0=ps,
                                        scalar1=b1_sb[:, rc:rc + 1], scalar2=0.0,
                                        op0=ALU.add, op1=ALU.max)

            for jn in range(2):
                pss = [pf2.tile([P, 512], F32, tag="f2", name=f"psf2_{jn}_{t_}")
                       for t_ in range(TC)]
                for rc in range(RC):
                    w2_t = wstream.tile([P, 512], BF16, tag="w2", name=f"w2_{jn}_{rc}")
                    nc.sync.dma_start(out=w2_t, in_=w2_r[:, rc, jn * 512:(jn + 1) * 512])
                    for t_ in range(TC):
                        nc.tensor.matmul(pss[t_], lhsT=h1T[:, rc, t_ * P:(t_ + 1) * P],
                                         rhs=w2_t, start=(rc == 0), stop=(rc == RC - 1))
                for t_ in range(TC):
                    o_t = out_pool.tile([P, 512], F32, tag="outp", name=f"o_{jn}_{t_}")
                    nc.vector.tensor_tensor(out=o_t, in0=pss[t_],
                                            in1=x_own[:, t_, jn * 512:(jn + 1) * 512],
                                            op=ALU.add)
                    nc.sync.dma_start(
                        out=out[t_ * P:(t_ + 1) * P, jn * 512:(jn + 1) * 512],
                        in_=o_t)

    nc.compile()
    return nc


_CACHE = {}


def _get_runner():
    """Build the Bass program once and return a cached executor.

    The executor maps a list of 8 per-core input dicts to a list of 8
    per-core output dicts, running the compiled NEFF on the 8 NeuronCores
    via PJRT/shard_map (same mechanism as bass2jax.run_bass_via_pjrt, but
    with the jitted callable cached so repeat calls don't recompile).
    """
    if "runner" in _CACHE:
        return _CACHE["runner"]

    import jax
    import jax.numpy as jnp
    from jax.experimental.shard_map import shard_map
    from jax.sharding import Mesh, PartitionSpec
    from concourse import bass2jax

    nc = _build_nc()
    bass2jax.install_neuronx_cc_hook()

    partition_name = (nc.partition_id_tensor.name
                      if nc.partition_id_tensor is not None else None)
    in_names, out_names, out_avals, zero_outs = [], [], [], []
    for alloc in nc.m.functions[0].allocations:
        if not isinstance(alloc, mybir.MemoryLocationSet):
            continue
        name = alloc.memorylocations[0].name
        if alloc.kind == "ExternalInput":
            if name != partition_name:
                in_names.append(name)
        elif alloc.kind == "ExternalOutput":
            out_names.append(name)
            shape = tuple(alloc.tensor_shape)
            dtype = mybir.dt.np(alloc.dtype)
            out_avals.append(jax.core.ShapedArray(shape, dtype))
            zero_outs.append(np.zeros(shape, dtype))
    n_params = len(in_names)
    n_outs = len(out_names)
    all_in_names = in_names + out_names
    if partition_name is not None:
        all_in_names = all_in_names + [partition_name]

    def _body_reps(reps):
        def _body(*args):
            ins = list(args[:n_params])
            outs = list(args[n_params:])
            extra = ([bass2jax.partition_id_tensor()]
                     if partition_name is not None else [])
            for _ in range(reps):
                outs = list(bass2jax._bass_exec_p.bind(
                    *ins, *outs, *extra,
                    out_avals=tuple(out_avals),
                    in_names=tuple(all_in_names),
                    out_names=tuple(out_names),
                    lowering_input_output_aliases=(),
                    sim_require_finite=False,
                    sim_require_nnan=False,
                    nc=nc,
                ))
            return tuple(outs)
        return _body

    devices = jax.devices()[:NCORES]
    mesh = Mesh(np.asarray(devices), ("core",))
    specs = (PartitionSpec("core"),) * (n_params + n_outs)
    out_specs = (PartitionSpec("core"),) * n_outs

    jitted = {}

    def get_jitted(reps):
        if reps not in jitted:
            jitted[reps] = jax.jit(shard_map(
                _body_reps(reps), mesh=mesh, in_specs=specs,
                out_specs=out_specs, check_rep=False), keep_unused=True)
        return jitted[reps]

    class Runner:
        nc_obj = nc

        def get_jitted(self, reps):
            return get_jitted(reps)

        def prepare(self, in_maps):
            """Concatenate per-core inputs to global arrays."""
            concat_in = [
                np.concatenate([np.asarray(in_maps[c][nm]) for c in range(NCORES)],
                               axis=0)
                for nm in in_names
            ]
            concat_zeros = [
                np.zeros((NCORES * z.shape[0], *z.shape[1:]), z.dtype)
                for z in zero_outs
            ]
            return concat_in + concat_zeros

        def execute(self, prepared, reps=1):
            out_arrs = get_jitted(reps)(*prepared)
            jax.block_until_ready(out_arrs)
            return out_arrs

        def split(self, out_arrs):
            return [
                {nm: np.asarray(out_arrs[i]).reshape(NCORES, *out_avals[i].shape)[c]
                 for i, nm in enumerate(out_names)}
                for c in range(NCORES)
            ]

        def __call__(self, in_maps):
            return self.split(self.execute(self.prepare(in_maps)))

    runner = Runner()
    _CACHE["runner"] = runner
    return runner


def make_in_maps(x, w_q, w_k, w_v, w_o, ln1_g, ln1_b, ln2_g, ln2_b,
                 w1, b1, w2, b2):
    x = np.asarray(x, dtype=np.float32)
    shared = {
        "wqT": np.ascontiguousarray(np.asarray(w_q, np.float32).T).astype(BF_NP),
        "wkT": np.ascontiguousarray(np.asarray(w_k, np.float32).T).astype(BF_NP),
        "wvT": np.ascontiguousarray(np.asarray(w_v, np.float32).T).astype(BF_NP),
        "woT": np.ascontiguousarray(np.asarray(w_o, np.float32).T).astype(BF_NP),
        "w1": np.asarray(w1, np.float32).astype(BF_NP),
        "w2": np.asarray(w2, np.float32).astype(BF_NP),
        "ln1g": np.asarray(ln1_g, np.float32),
        "ln1b": np.asarray(ln1_b, np.float32),
        "ln2g": np.asarray(ln2_g, np.float32),
        "ln2b": np.asarray(ln2_b, np.float32),
        "b1": np.asarray(b1, np.float32),
        "b2": np.asarray(b2, np.float32),
    }
    in_maps = []
    for c in range(NCORES):
        b, c4 = c // 4, c % 4
        xb_c = np.ascontiguousarray(np.roll(x[b], -T * c4, axis=0))
        in_maps.append({"xb": xb_c, **shared})
    return in_maps


def kernel(x, src_mask, w_q, w_k, w_v, w_o, ln1_g, ln1_b, ln2_g, ln2_b,
           w1, b1, w2, b2):
    """Full-input entry point: returns the [B, S, D] float32 output."""
    runner = _get_runner()
    in_maps = make_in_maps(x, w_q, w_k, w_v, w_o, ln1_g, ln1_b, ln2_g,
                           ln2_b, w1, b1, w2, b2)
    results = runner(in_maps)
    out = np.empty((B, S, D), dtype=np.float32)
    for c in range(NCORES):
        b, c4 = c // 4, c % 4
        out[b, T * c4:T * (c4 + 1), :] = results[c]["out"]
    return out


# revision 5
# speedup vs baseline: 6125.0330x; 6125.0330x over previous
"""Trainium2 Bass kernel for a pre-norm transformer encoder block.

Reference computation (per batch):
    x = x + MHA(LN1(x));  x = x + FFN(LN2(x))
with B=2, S=2048, D=1024, H=16 heads (HD=64), HID=4096, fp32 params,
src_mask all-ones (no-op).

Sharding: pure data parallel over the 8 NeuronCores. Core c handles batch
b = c // 4 and query-token chunk c % 4 (512 tokens). Each core recomputes
K/V for its full batch (4x redundant) so no collectives are needed. The
batch rows are rolled on the host so each core's own tokens are rows 0:512;
attention is permutation-invariant over keys so rolling is safe.

On-chip math: all GEMMs in bf16 with fp32 PSUM accumulation; layernorm,
softmax statistics and residuals in fp32. Softmax is computed in
"transposed score" space (keys on partitions) so no P-matrix transposes
are needed; the row sums come from an extra all-ones column appended to V,
and the 1/sum normalization is applied to the attention output via a
DRAM-bounce broadcast.
"""

import numpy as np
import ml_dtypes

import concourse.bacc as bacc
import concourse.bass as bass
import concourse.mybir as mybir
import concourse.tile as tile
from concourse.masks import make_identity

P = 128
B, S, D, H, HD, HID = 2, 2048, 1024, 16, 64, 4096
T = 512                     # own query tokens per core
DC = D // P                 # 8  d-chunks
SC = S // P                 # 16 key-chunks
TC = T // P                 # 4  own-token chunks
RC = HID // P               # 32 hidden chunks
NCORES = 8
EPS = 1e-5

F32 = mybir.dt.float32
BF16 = mybir.dt.bfloat16
AF = mybir.ActivationFunctionType
ALU = mybir.AluOpType
BF_NP = ml_dtypes.bfloat16


def _build_nc():
    nc = bacc.Bacc("TRN2", target_bir_lowering=False, debug=False)

    xb = nc.declare_dram_parameter("xb", [S, D], F32, isOutput=False)
    wqT = nc.declare_dram_parameter("wqT", [D, D], BF16, isOutput=False)
    wkT = nc.declare_dram_parameter("wkT", [D, D], BF16, isOutput=False)
    wvT = nc.declare_dram_parameter("wvT", [D, D], BF16, isOutput=False)
    woT = nc.declare_dram_parameter("woT", [D, D], BF16, isOutput=False)
    w1 = nc.declare_dram_parameter("w1", [D, HID], BF16, isOutput=False)
    w2 = nc.declare_dram_parameter("w2", [HID, D], BF16, isOutput=False)
    ln1g = nc.declare_dram_parameter("ln1g", [D], F32, isOutput=False)
    ln1b = nc.declare_dram_parameter("ln1b", [D], F32, isOutput=False)
    ln2g = nc.declare_dram_parameter("ln2g", [D], F32, isOutput=False)
    ln2b = nc.declare_dram_parameter("ln2b", [D], F32, isOutput=False)
    b1 = nc.declare_dram_parameter("b1", [HID], F32, isOutput=False)
    b2 = nc.declare_dram_parameter("b2", [D], F32, isOutput=False)
    out = nc.declare_dram_parameter("out", [T, D], F32, isOutput=True)

    sums_dram = nc.dram_tensor("sums_dram", [H, T], F32)
    recip_dram = nc.dram_tensor("recip_dram", [H, T], F32)

    # DRAM views used for strided loads
    wqT_r = wqT[:, :].rearrange("(dc p) o -> p dc o", p=P)
    wkT_r = wkT[:, :].rearrange("(dc p) o -> p dc o", p=P)
    wvT_r = wvT[:, :].rearrange("(dc p) o -> p dc o", p=P)
    woT_r = woT[:, :].rearrange("(dc p) o -> p dc o", p=P)
    w1_r = w1[:, :].rearrange("(dc p) r -> p dc r", p=P)
    w2_r = w2[:, :].rearrange("(rc p) d -> p rc d", p=P)

    def bcast_rows(src_ap, nrows):
        return bass.AP(tensor=src_ap.tensor, offset=src_ap.offset,
                       ap=[[0, nrows], *src_ap.ap[1:]])

    import contextlib
    with tile.TileContext(nc) as tc, contextlib.ExitStack() as ctx:
        consts = ctx.enter_context(tc.tile_pool(name="consts", bufs=1))
        persist = ctx.enter_context(tc.tile_pool(name="persist", bufs=1))
        poolA = ctx.enter_context(tc.tile_pool(name="poolA", bufs=1))
        poolB = ctx.enter_context(tc.tile_pool(name="poolB", bufs=1))
        small = ctx.enter_context(tc.tile_pool(name="small", bufs=4))
        xb_pool = ctx.enter_context(tc.tile_pool(name="xb_pool", bufs=2))
        wsmall = ctx.enter_context(tc.tile_pool(name="wsmall", bufs=2))
        wbig = ctx.enter_context(tc.tile_pool(name="wbig", bufs=1))
        wstream = ctx.enter_context(tc.tile_pool(name="wstream", bufs=3))
        exp_pool = ctx.enter_context(tc.tile_pool(name="exp_pool", bufs=6))
        sums_pool = ctx.enter_context(tc.tile_pool(name="sums_pool", bufs=2))
        rdup_pool = ctx.enter_context(tc.tile_pool(name="rdup_pool", bufs=2))
        out_pool = ctx.enter_context(tc.tile_pool(name="out_pool", bufs=2))

        # ---------------- constants ----------------
        identity = consts.tile([P, P], BF16)
        make_identity(nc, identity)
        eps_t = consts.tile([P, 1], F32)
        nc.vector.memset(eps_t, EPS)
        g1_sb = consts.tile([P, DC], F32)
        nc.sync.dma_start(out=g1_sb, in_=ln1g[:].rearrange("(c p) -> p c", p=P))
        b1ln_sb = consts.tile([P, DC], F32)
        nc.sync.dma_start(out=b1ln_sb, in_=ln1b[:].rearrange("(c p) -> p c", p=P))
        g2_sb = consts.tile([P, DC], F32)
        nc.sync.dma_start(out=g2_sb, in_=ln2g[:].rearrange("(c p) -> p c", p=P))
        b2ln_sb = consts.tile([P, DC], F32)
        nc.sync.dma_start(out=b2ln_sb, in_=ln2b[:].rearrange("(c p) -> p c", p=P))
        b1_sb = consts.tile([P, RC], F32)
        nc.sync.dma_start(out=b1_sb, in_=b1[:].rearrange("(c p) -> p c", p=P))
        b2rep = consts.tile([P, D], F32)
        nc.sync.dma_start(out=b2rep, in_=bcast_rows(b2[:].rearrange("(one d) -> one d", one=1), P))

        # ---------------- persistent tensors ----------------
        x_own = persist.tile([P, TC, D], F32)       # own x rows; becomes x2 in place
        QT = persist.tile([P, DC, T], BF16)
        KT = persist.tile([P, DC, S], BF16)
        attnT = persist.tile([P, DC, T], BF16)
        hT = persist.tile([P, DC, T], BF16)
        h_bf = persist.tile([P, TC, D], BF16)
        xn_bf = poolA.tile([P, SC, D], BF16, tag="sharedA", name="xn_bf")

        def layernorm_chunk(src, dst_bf, g_unused=None):
            """src [P, D] f32 -> dst_bf [P, D] bf16 normalized (no gamma/beta)."""
            stats = small.tile([P, 2, 6], F32, tag="stats", name="stats")
            nc.vector.bn_stats(out=stats[:, 0, :], in_=src[:, 0:512])
            nc.vector.bn_stats(out=stats[:, 1, :], in_=src[:, 512:1024])
            mv = small.tile([P, 2], F32, tag="mv", name="mv")
            nc.vector.bn_aggr(out=mv, in_=stats)
            std = small.tile([P, 1], F32, tag="std", name="std")
            nc.scalar.activation(out=std, in_=mv[:, 1:2], func=AF.Sqrt, bias=eps_t)
            rstd = small.tile([P, 1], F32, tag="rstd", name="rstd")
            nc.vector.reciprocal(out=rstd, in_=std)
            nc.vector.tensor_scalar(out=dst_bf, in0=src, scalar1=mv[:, 0:1],
                                    scalar2=rstd, op0=ALU.subtract, op1=ALU.mult)

        # ================ Phase 1: LN1 + transpose + QKV ================
        with tc.tile_pool(name="pt", bufs=2, space="PSUM") as pt, \
             tc.tile_pool(name="pq", bufs=3, space="PSUM") as pq:

            # LN1 over the full batch (16 chunks); own rows also kept in f32
            for t in range(SC):
                if t < TC:
                    xt = x_own[:, t, :]
                else:
                    xtile = xb_pool.tile([P, D], F32, tag="xb", name=f"xb_{t}")
                    xt = xtile
                nc.sync.dma_start(out=xt, in_=xb[t * P:(t + 1) * P, :])
                layernorm_chunk(xt, xn_bf[:, t, :])

            # transpose xn -> xnT, applying ln1 gamma/beta on the drain
            xnT = poolB.tile([P, DC, S], BF16, tag="sharedB", name="xnT")
            for dc in range(DC):
                for tq in range(SC // 4):
                    ps_t = pt.tile([P, 4, P], BF16, tag="tp", name=f"tp_{dc}_{tq}")
                    for i in range(4):
                        nc.tensor.transpose(
                            ps_t[:, i, :], xn_bf[:, tq * 4 + i, dc * P:(dc + 1) * P],
                            identity)
                    dst = xnT[:, dc, tq * 512:(tq + 1) * 512]
                    nc.vector.tensor_scalar(
                        out=dst.rearrange("p (i c) -> p i c", i=4),
                        in0=ps_t, scalar1=g1_sb[:, dc:dc + 1],
                        scalar2=b1ln_sb[:, dc:dc + 1],
                        op0=ALU.mult, op1=ALU.add)

            # Q^T projection: [P, DC, T]
            for oc in range(DC):
                wq_t = wsmall.tile([P, DC, P], BF16, tag="wq", name=f"wq_{oc}")
                nc.sync.dma_start(out=wq_t, in_=wqT_r[:, :, oc * P:(oc + 1) * P])
                ps = pq.tile([P, T], F32, tag="qkv", name=f"psq_{oc}")
                for dc in range(DC):
                    nc.tensor.matmul(ps, lhsT=wq_t[:, dc, :], rhs=xnT[:, dc, 0:T],
                                     start=(dc == 0), stop=(dc == DC - 1))
                nc.scalar.copy(out=QT[:, oc, :], in_=ps)

            # K^T projection: [P, DC, S]
            for oc in range(DC):
                wk_t = wsmall.tile([P, DC, P], BF16, tag="wk", name=f"wk_{oc}")
                nc.sync.dma_start(out=wk_t, in_=wkT_r[:, :, oc * P:(oc + 1) * P])
                for nt in range(S // 512):
                    ps = pq.tile([P, 512], F32, tag="qkv", name=f"psk_{oc}_{nt}")
                    for dc in range(DC):
                        nc.tensor.matmul(
                            ps, lhsT=wk_t[:, dc, :],
                            rhs=xnT[:, dc, nt * 512:(nt + 1) * 512],
                            start=(dc == 0), stop=(dc == DC - 1))
                    nc.scalar.copy(out=KT[:, oc, nt * 512:(nt + 1) * 512], in_=ps)

            # V natural projection with ones column: [P, SC, H, HD+1]
            V_sb = poolA.tile([P, SC, H, HD + 1], BF16, tag="sharedA", name="V_sb")
            nc.vector.memset(V_sb[:, :, :, HD:HD + 1], 1.0)
            for jn in range(2):
                wv_t = wbig.tile([P, DC, 512], BF16, tag="wv", name=f"wv_{jn}")
                nc.sync.dma_start(out=wv_t, in_=wvT_r[:, :, jn * 512:(jn + 1) * 512])
                for sc in range(SC):
                    ps = pq.tile([P, 512], F32, tag="qkv", name=f"psv_{jn}_{sc}")
                    for dc in range(DC):
                        nc.tensor.matmul(
                            ps, lhsT=xnT[:, dc, sc * P:(sc + 1) * P],
                            rhs=wv_t[:, dc, :],
                            start=(dc == 0), stop=(dc == DC - 1))
                    nc.scalar.copy(
                        out=V_sb[:, sc, jn * 8:(jn + 1) * 8, 0:HD],
                        in_=ps.rearrange("p (h d) -> p h d", h=8))

        # ================ Phase 2: attention ================
        with tc.tile_pool(name="psc", bufs=4, space="PSUM") as psc, \
             tc.tile_pool(name="ppv", bufs=4, space="PSUM") as ppv:

            for p8 in range(H // 2):
                hA, hB = 2 * p8, 2 * p8 + 1
                pvA = ppv.tile([HD + 1, T], F32, tag="pv", name=f"pvA_{p8}")
                pvB = ppv.tile([HD + 1, T], F32, tag="pv", name=f"pvB_{p8}")
                for kc in range(SC):
                    sA = psc.tile([P, T], F32, tag="sc", name=f"sA_{p8}_{kc}")
                    sB = psc.tile([P, T], F32, tag="sc", name=f"sB_{p8}_{kc}")
                    nc.tensor.matmul(sA, lhsT=KT[0:64, p8, kc * P:(kc + 1) * P],
                                     rhs=QT[0:64, p8, :], start=True, stop=True,
                                     tile_position=(0, 0))
                    nc.tensor.matmul(sB, lhsT=KT[64:128, p8, kc * P:(kc + 1) * P],
                                     rhs=QT[64:128, p8, :], start=True, stop=True,
                                     tile_position=(64, 0))
                    eA = exp_pool.tile([P, T], BF16, tag="exp", name=f"eA_{p8}_{kc}")
                    eB = exp_pool.tile([P, T], BF16, tag="exp", name=f"eB_{p8}_{kc}")
                    nc.scalar.activation(out=eA, in_=sA, func=AF.Exp, scale=0.125)
                    nc.scalar.activation(out=eB, in_=sB, func=AF.Exp, scale=0.125)
                    nc.tensor.matmul(pvA, lhsT=V_sb[:, kc, hA, :], rhs=eA,
                                     start=(kc == 0), stop=(kc == SC - 1))
                    nc.tensor.matmul(pvB, lhsT=V_sb[:, kc, hB, :], rhs=eB,
                                     start=(kc == 0), stop=(kc == SC - 1))
                # sums row -> DRAM bounce; unnormalized attnT -> SBUF (bf16)
                smA = sums_pool.tile([1, T], F32, tag="sums", name=f"smA_{p8}")
                smB = sums_pool.tile([1, T], F32, tag="sums", name=f"smB_{p8}")
                nc.vector.tensor_copy(out=smA, in_=pvA[HD:HD + 1, :])
                nc.vector.tensor_copy(out=smB, in_=pvB[HD:HD + 1, :])
                nc.sync.dma_start(out=sums_dram[hA:hA + 1, :], in_=smA)
                nc.sync.dma_start(out=sums_dram[hB:hB + 1, :], in_=smB)
                nc.vector.tensor_copy(out=attnT[0:64, p8, :], in_=pvA[0:HD, :])
                nc.vector.tensor_copy(out=attnT[64:128, p8, :], in_=pvB[0:HD, :])

            # reciprocal of all head sums, then broadcast-normalize attnT
            sums16 = persist.tile([H, T], F32)
            nc.sync.dma_start(out=sums16, in_=sums_dram[:, :])
            recip16 = persist.tile([H, T], F32)
            nc.vector.reciprocal(out=recip16, in_=sums16)
            nc.sync.dma_start(out=recip_dram[:, :], in_=recip16)
            for p8 in range(H // 2):
                rd = rdup_pool.tile([P, T], F32, tag="rdup", name=f"rd_{p8}")
                nc.sync.dma_start(out=rd[0:64, :],
                                  in_=bcast_rows(recip_dram[2 * p8:2 * p8 + 1, :], 64))
                nc.sync.dma_start(out=rd[64:128, :],
                                  in_=bcast_rows(recip_dram[2 * p8 + 1:2 * p8 + 2, :], 64))
                nc.vector.tensor_tensor(out=attnT[:, p8, :], in0=attnT[:, p8, :],
                                        in1=rd, op=ALU.mult)

        # ================ Phase 3: output proj + residual + LN2 ================
        with tc.tile_pool(name="po", bufs=5, space="PSUM") as po, \
             tc.tile_pool(name="pt2", bufs=2, space="PSUM") as pt2:

            for jn in range(2):
                pss = [po.tile([P, 512], F32, tag="o", name=f"pso_{jn}_{t_}")
                       for t_ in range(TC)]
                for ic in range(DC):
                    wo_t = wstream.tile([P, 512], BF16, tag="wo", name=f"wo_{jn}_{ic}")
                    nc.sync.dma_start(out=wo_t, in_=woT_r[:, ic, jn * 512:(jn + 1) * 512])
                    for t_ in range(TC):
                        nc.tensor.matmul(pss[t_], lhsT=attnT[:, ic, t_ * P:(t_ + 1) * P],
                                         rhs=wo_t, start=(ic == 0), stop=(ic == DC - 1))
                for t_ in range(TC):
                    sl = x_own[:, t_, jn * 512:(jn + 1) * 512]
                    nc.vector.tensor_tensor(out=sl, in0=pss[t_], in1=sl, op=ALU.add)

            # LN2 (x_own now holds x2); h_bf bf16 normalized
            for t_ in range(TC):
                layernorm_chunk(x_own[:, t_, :], h_bf[:, t_, :])
                # after LN2 consumed the chunk, fold b2 into the residual base
                nc.vector.tensor_tensor(out=x_own[:, t_, :], in0=x_own[:, t_, :],
                                        in1=b2rep, op=ALU.add)

            # transpose h -> hT with ln2 gamma/beta fused on the drain
            for dc in range(DC):
                ps_t = pt2.tile([P, TC, P], BF16, tag="tp2", name=f"tp2_{dc}")
                for i in range(TC):
                    nc.tensor.transpose(
                        ps_t[:, i, :], h_bf[:, i, dc * P:(dc + 1) * P], identity)
                nc.vector.tensor_scalar(
                    out=hT[:, dc, :].rearrange("p (i c) -> p i c", i=TC),
                    in0=ps_t, scalar1=g2_sb[:, dc:dc + 1],
                    scalar2=b2ln_sb[:, dc:dc + 1],
                    op0=ALU.mult, op1=ALU.add)

        # ================ Phase 4: FFN ================
        with tc.tile_pool(name="pf1", bufs=3, space="PSUM") as pf1, \
             tc.tile_pool(name="pf2", bufs=5, space="PSUM") as pf2:

            h1T = poolB.tile([P, RC, T], BF16, tag="sharedB", name="h1T")
            for rc in range(RC):
                w1_t = wsmall.tile([P, DC, P], BF16, tag="w1", name=f"w1_{rc}")
                nc.sync.dma_start(out=w1_t, in_=w1_r[:, :, rc * P:(rc + 1) * P])
                ps = pf1.tile([P, T], F32, tag="f1", name=f"psf1_{rc}")
                for dc in range(DC):
                    nc.tensor.matmul(ps, lhsT=w1_t[:, dc, :], rhs=hT[:, dc, :],
                                     start=(dc == 0), stop=(dc == DC - 1))
                # relu(x + b1) fused on the drain
                nc.vector.tensor_scalar(out=h1T[:, rc, :], in0=ps,
                                        scalar1=b1_sb[:, rc:rc + 1], scalar2=0.0,
                                        op0=ALU.add, op1=ALU.max)

            for jn in range(2):
                pss = [pf2.tile([P, 512], F32, tag="f2", name=f"psf2_{jn}_{t_}")
                       for t_ in range(TC)]
                for rc in range(RC):
                    w2_t = wstream.tile([P, 512], BF16, tag="w2", name=f"w2_{jn}_{rc}")
                    nc.sync.dma_start(out=w2_t, in_=w2_r[:, rc, jn * 512:(jn + 1) * 512])
                    for t_ in range(TC):
                        nc.tensor.matmul(pss[t_], lhsT=h1T[:, rc, t_ * P:(t_ + 1) * P],
                                         rhs=w2_t, start=(rc == 0), stop=(rc == RC - 1))
                for t_ in range(TC):
                    o_t = out_pool.tile([P, 512], F32, tag="outp", name=f"o_{jn}_{t_}")
                    nc.vector.tensor_tensor(out=o_t, in0=pss[t_],
                                            in1=x_own[:, t_, jn * 512:(jn + 1) * 512],
                                            op=ALU.add)
                    nc.sync.dma_start(
                        out=out[t_ * P:(t_ + 1) * P, jn * 512:(jn + 1) * 512],
                        in_=o_t)

    nc.compile()
    return nc


_CACHE = {}


def _get_runner():
    """Build the Bass program once and return a cached executor.

    The executor maps a list of 8 per-core input dicts to a list of 8
    per-core output dicts, running the compiled NEFF on the 8 NeuronCores
    via PJRT/shard_map (same mechanism as bass2jax.run_bass_via_pjrt, but
    with the jitted callable cached so repeat calls don't recompile).
    """
    if "runner" in _CACHE:
        return _CACHE["runner"]

    import jax
    import jax.numpy as jnp
    from jax.experimental.shard_map import shard_map
    from jax.sharding import Mesh, PartitionSpec
    from concourse import bass2jax

    nc = _build_nc()
    bass2jax.install_neuronx_cc_hook()

    partition_name = (nc.partition_id_tensor.name
                      if nc.partition_id_tensor is not None else None)
    in_names, out_names, out_avals, zero_outs = [], [], [], []
    for alloc in nc.m.functions[0].allocations:
        if not isinstance(alloc, mybir.MemoryLocationSet):
            continue
        name = alloc.memorylocations[0].name
        if alloc.kind == "ExternalInput":
            if name != partition_name:
                in_names.append(name)
        elif alloc.kind == "ExternalOutput":
            out_names.append(name)
            shape = tuple(alloc.tensor_shape)
            dtype = mybir.dt.np(alloc.dtype)
            out_avals.append(jax.core.ShapedArray(shape, dtype))
            zero_outs.append(np.zeros(shape, dtype))
    n_params = len(in_names)
    n_outs = len(out_names)
    all_in_names = in_names + out_names
    if partition_name is not None:
        all_in_names = all_in_names + [partition_name]

    def _body_reps(reps):
        def _body(*args):
            ins = list(args[:n_params])
            outs = list(args[n_params:])
            extra = ([bass2jax.partition_id_tensor()]
                     if partition_name is not None else [])
            for _ in range(reps):
                outs = list(bass2jax._bass_exec_p.bind(
                    *ins, *outs, *extra,
                    out_avals=tuple(out_avals),
                    in_names=tuple(all_in_names),
                    out_names=tuple(out_names),
                    lowering_input_output_aliases=(),
                    sim_require_finite=False,
                    sim_require_nnan=False,
                    nc=nc,
                ))
            return tuple(outs)
        return _body

    devices = jax.devices()[:NCORES]
    mesh = Mesh(np.asarray(devices), ("core",))
    specs = (PartitionSpec("core"),) * (n_params + n_outs)
    out_specs = (PartitionSpec("core"),) * n_outs

    jitted = {}

    def get_jitted(reps):
        if reps not in jitted:
            jitted[reps] = jax.jit(shard_map(
                _body_reps(reps), mesh=mesh, in_specs=specs,
                out_specs=out_specs, check_rep=False), keep_unused=True)
        return jitted[reps]

    class Runner:
        nc_obj = nc

        def get_jitted(self, reps):
            return get_jitted(reps)

        def prepare(self, in_maps, device=False):
            """Concatenate per-core inputs to global arrays."""
            concat_in = [
                np.concatenate([np.asarray(in_maps[c][nm]) for c in range(NCORES)],
                               axis=0)
                for nm in in_names
            ]
            concat_zeros = [
                np.zeros((NCORES * z.shape[0], *z.shape[1:]), z.dtype)
                for z in zero_outs
            ]
            args = concat_in + concat_zeros
            if device:
                from jax.sharding import NamedSharding
                sh = NamedSharding(mesh, PartitionSpec("core"))
                args = [jax.device_put(a, sh) for a in args]
                jax.block_until_ready(args)
            return args

        def execute(self, prepared, reps=1):
            out_arrs = get_jitted(reps)(*prepared)
            jax.block_until_ready(out_arrs)
            return out_arrs

        def split(self, out_arrs):
            return [
                {nm: np.asarray(out_arrs[i]).reshape(NCORES, *out_avals[i].shape)[c]
                 for i, nm in enumerate(out_names)}
                for c in range(NCORES)
            ]

        def __call__(self, in_maps):
            return self.split(self.execute(self.prepare(in_maps)))

    runner = Runner()
    _CACHE["runner"] = runner
    return runner


def make_in_maps(x, w_q, w_k, w_v, w_o, ln1_g, ln1_b, ln2_g, ln2_b,
                 w1, b1, w2, b2):
    x = np.asarray(x, dtype=np.float32)
    shared = {
        "wqT": np.ascontiguousarray(np.asarray(w_q, np.float32).T).astype(BF_NP),
        "wkT": np.ascontiguousarray(np.asarray(w_k, np.float32).T).astype(BF_NP),
        "wvT": np.ascontiguousarray(np.asarray(w_v, np.float32).T).astype(BF_NP),
        "woT": np.ascontiguousarray(np.asarray(w_o, np.float32).T).astype(BF_NP),
        "w1": np.asarray(w1, np.float32).astype(BF_NP),
        "w2": np.asarray(w2, np.float32).astype(BF_NP),
        "ln1g": np.asarray(ln1_g, np.float32),
        "ln1b": np.asarray(ln1_b, np.float32),
        "ln2g": np.asarray(ln2_g, np.float32),
        "ln2b": np.asarray(ln2_b, np.float32),
        "b1": np.asarray(b1, np.float32),
        "b2": np.asarray(b2, np.float32),
    }
    in_maps = []
    for c in range(NCORES):
        b, c4 = c // 4, c % 4
        xb_c = np.ascontiguousarray(np.roll(x[b], -T * c4, axis=0))
        in_maps.append({"xb": xb_c, **shared})
    return in_maps


def kernel(x, src_mask, w_q, w_k, w_v, w_o, ln1_g, ln1_b, ln2_g, ln2_b,
           w1, b1, w2, b2):
    """Full-input entry point: returns the [B, S, D] float32 output."""
    runner = _get_runner()
    in_maps = make_in_maps(x, w_q, w_k, w_v, w_o, ln1_g, ln1_b, ln2_g,
                           ln2_b, w1, b1, w2, b2)
    results = runner(in_maps)
    out = np.empty((B, S, D), dtype=np.float32)
    for c in range(NCORES):
        b, c4 = c // 4, c % 4
        out[b, T * c4:T * (c4 + 1), :] = results[c]["out"]
    return out
